# revision 10
# baseline (speedup 1.0000x reference)
"""Causal attention (B=4, L=2048, d_model=1024, d_k=d_v=128) on 8 TRN2 NeuronCores.

Sharding (SPMD — one program, per-core data):
  core c -> batch b = c//2, parity par = c%2.
  Core handles q-blocks j = 2k+par for slot k in 0..7 (128 rows each).
  X^T's column blocks are split by parity into two slot-ordered inputs
  (xq = own query-parity blocks, xo = other parity), stored HOST-SIDE in
  piece-major [group, partition, chunk, col] layout so every input DMA is
  fully contiguous per partition (8KB runs).  Slot k attends key-slots
  0..k of EACH parity; the causal boundary is uniform (diagonal mask on
  own-parity key-slot m == k; other-parity m == k all-or-nothing by core
  parity) and is applied as a multiplicative 0/1 bf16 mask AFTER exp.

Within a core (all matmuls contract on the partition dim):
  - Projections: weight-stationary, one N=512 matmul per d_model chunk
    per 512-column group (no narrow pieces -> fewer LDWEIGHTS).
  - Scores TRANSPOSED: S^T[key, q] = K^T_blk.T @ Q^T; exp() writes A^T
    straight to SBUF (bf16).  Masking multiplies the diagonal q-slot
    after exp (scores bounded ~|12|, exp finite, so 0-mult is exact).
  - V transposes are quad-batched: 4 PE transposes into one PSUM bank,
    one strided copy out to a [128, 4, 129] tile whose col 128 is ones.
  - AV streams: as each A^T block lands, it is accumulated into the
    per-slot Z PSUM accumulators (8 slots packed 3-per-bank, 129 cols
    each: col 128 is the softmax denominator via the ones column).
    Slot k closes on (parity 1, m == k); outputs drain in two batched
    DMAs of 4 slots each.
"""

import os
import sys

sys.path.insert(0, "/opt/trn_rl_repo")
sys.path.insert(0, "/opt/trn_rl_repo/concourse")

import ml_dtypes
import numpy as np

import concourse.bass as bass  # noqa: F401
import concourse.mybir as mybir
import concourse.tile as tile
from concourse import bacc
from concourse.bass_utils import run_bass_kernel_spmd
from concourse.masks import make_identity

B, L, DM, DK, DV = 4, 2048, 1024, 128, 128
NB = L // 128    # 16 key blocks per batch
SLOTS = 8        # q-blocks per core
NCH = DM // 128  # 8 d_model chunks
G = 2            # q-slot groups of 4 (512 cols)
SCALE = float(DK) ** -0.5

COMPUTE = os.environ.get("ATTN_COMPUTE", "bf16")  # "bf16" | "f32"

F32 = mybir.dt.float32


def _cdt():
    return mybir.dt.bfloat16 if COMPUTE == "bf16" else mybir.dt.float32


def _np_cdt():
    return ml_dtypes.bfloat16 if COMPUTE == "bf16" else np.float32


def build_nc():
    cdt = _cdt()
    nc = bacc.Bacc()

    xq_ext = nc.declare_dram_parameter("xq", [G, 128, NCH * 512], cdt, isOutput=False)
    xo_ext = nc.declare_dram_parameter("xo", [G, 128, NCH * 512], cdt, isOutput=False)
    # weights pre-arranged on host to the SBUF chunk layout
    # [p, c*128+d] = W[c*128+p, d] so the DMA is fully contiguous
    wq_ext = nc.declare_dram_parameter("wq", [128, DM], cdt, isOutput=False)
    wk_ext = nc.declare_dram_parameter("wk", [128, DM], cdt, isOutput=False)
    wv_ext = nc.declare_dram_parameter("wv", [128, DM], cdt, isOutput=False)
    # multiplicative post-exp masks: [key 128, 2*128 q] — col block 0 is the
    # diagonal triangle (own parity), col block 1 all-0/all-1 (other parity)
    mask_ext = nc.declare_dram_parameter("maskT", [128, 256], cdt, isOutput=False)
    out_ext = nc.declare_dram_parameter("out", [SLOTS * 128, DV], F32, isOutput=True)

    xq_r = xq_ext.rearrange("g p (c j) -> g p c j", j=512)
    xo_r = xo_ext.rearrange("g p (c j) -> g p c j", j=512)

    with tile.TileContext(nc) as tc:
        with (
            tc.tile_pool(name="persist", bufs=1) as persist,
            tc.tile_pool(name="mm_ps", bufs=5, space="PSUM") as mm_ps,
            tc.tile_pool(name="z_ps", bufs=1, space="PSUM") as z_ps,
            tc.tile_pool(name="work", bufs=4) as work,
        ):
            ident = persist.tile([128, 128], cdt, tag="ident")
            make_identity(nc, ident)

            # ---- DMA issue (sync queue), in consumption order ----
            w_sb = {}

            def load_w(name, ext):
                t = persist.tile([128, NCH, 128], cdt, tag=name, name=name)
                nc.sync.dma_start(out=t[:], in_=ext.rearrange("p (c d) -> p c d", d=128))
                w_sb[name] = t

            def load_x(r, g, nm, split=False):
                # returns list of (tile, base_chunk)
                if not split:
                    t = persist.tile([128, NCH, 512], cdt, tag=nm, name=nm)
                    nc.sync.dma_start(out=t[:], in_=r[g])
                    return [(t, 0)]
                parts = []
                for h in range(2):
                    t = persist.tile([128, NCH // 2, 512], cdt,
                                     tag=f"{nm}_{h}", name=f"{nm}_{h}")
                    nc.sync.dma_start(
                        out=t[:], in_=r[g][:, h * (NCH // 2):(h + 1) * (NCH // 2), :]
                    )
                    parts.append((t, h * (NCH // 2)))
                return parts

            load_w("wq", wq_ext)
            xq_sb = [load_x(xq_r, 0, "xq0", split=True), None]
            load_w("wk", wk_ext)
            load_w("wv", wv_ext)
            xq_sb[1] = load_x(xq_r, 1, "xq1")
            mask_sb = persist.tile([128, 256], cdt, tag="mask")
            nc.sync.dma_start(out=mask_sb[:], in_=mask_ext[:])
            xo_sb = [load_x(xo_r, 0, "xo0"), load_x(xo_r, 1, "xo1")]

            # ---- persistent SBUF tiles ----
            qt = [persist.tile([128, 512], cdt, tag=f"qt{g}", name=f"qt{g}")
                  for g in range(G)]
            kt = {(s, g): persist.tile([128, 512], cdt, tag=f"kt{s}{g}",
                                       name=f"kt{s}{g}")
                  for s in range(2) for g in range(G)}
            vt = {(s, g): persist.tile([128, 512], cdt, tag=f"vt{s}{g}",
                                       name=f"vt{s}{g}")
                  for s in range(2) for g in range(G)}
            # V in [key, v] layout, 4 blocks per tile, col 128 = ones
            v_quad = {}
            for s in range(2):
                for g in range(G):
                    t = persist.tile([128, 4, DV + 1], cdt, tag=f"vq{s}{g}",
                                     name=f"vq{s}{g}")
                    nc.vector.memset(t[:, :, DV:DV + 1], 1.0)
                    v_quad[(s, g)] = t
            at = {}
            for s in range(2):
                for m in range(SLOTS):
                    for g in range(G):
                        if m <= 4 * g + 3:
                            at[(s, m, g)] = persist.tile(
                                [128, 512], cdt, tag=f"at{s}_{m}_{g}",
                                name=f"at{s}_{m}_{g}")

            # z accumulators: 8 slots x [128, 129] f32 packed 3-per-bank.
            # A matmul with start=True aborts the bank's open accumulation
            # group, so slots sharing a bank must never use start: pre-zero
            # the banks and make every AV matmul a pure accumulate.
            zbank = []
            for i in range(3):
                t = z_ps.tile([128, 512], F32, tag=f"zb{i}", name=f"zb{i}")
                nc.vector.memset(t[:], 0.0)
                zbank.append(t)

            def zslice(k):
                b, o = k // 3, (k % 3) * (DV + 1)
                return zbank[b][:, o:o + DV + 1]

            # batched output staging: [128, 4 slots * 128]
            z_out = [persist.tile([128, 4, DV], F32, tag=f"zo{g}", name=f"zo{g}")
                     for g in range(G)]

            # ---- stages ----
            def proj(name, src, dst, g, kind):
                w = w_sb[name]
                ps = mm_ps.tile([128, 512], F32, tag="mm", name=f"p_{name}{g}")
                for t, base in src:
                    nch = t.shape[1]
                    for i in range(nch):
                        c = base + i
                        nc.tensor.matmul(
                            ps[:], w[:, c, :], t[:, i, :],
                            start=(c == 0), stop=(c == NCH - 1),
                        )
                if kind == "q":
                    nc.scalar.activation(
                        dst[:], ps[:], mybir.ActivationFunctionType.Copy,
                        bias=0.0, scale=SCALE,
                    )
                else:
                    nc.vector.tensor_copy(dst[:], ps[:])

            def vtrans(s, g):
                tq = mm_ps.tile([128, 4, 128], cdt, tag="mm", name=f"tq{s}{g}")
                for i in range(4):
                    nc.tensor.transpose(
                        tq[:, i, :], vt[(s, g)][:, i * 128:(i + 1) * 128], ident[:]
                    )
                nc.vector.tensor_copy(v_quad[(s, g)][:, :, 0:DV], tq[:])

            def scores(s, ms, g):
                for m in ms:
                    a = max(m - 4 * g, 0)
                    st = mm_ps.tile([128, 512], F32, tag="mm", name=f"st{s}{m}{g}")
                    nc.tensor.matmul(
                        st[:, a * 128:512],
                        kt[(s, m // 4)][:, (m % 4) * 128:(m % 4 + 1) * 128],
                        qt[g][:, a * 128:512],
                        start=True, stop=True,
                        skip_group_check=True,
                    )
                    nc.scalar.activation(
                        at[(s, m, g)][:, a * 128:512],
                        st[:, a * 128:512],
                        mybir.ActivationFunctionType.Exp,
                        bias=0.0, scale=1.0,
                    )
                    if 4 * g <= m <= 4 * g + 3:
                        # multiplicative causal mask on the idle gpsimd
                        # engine, keeping DVE free for psum casts
                        qo = (m - 4 * g) * 128
                        sl = at[(s, m, g)][:, qo:qo + 128]
                        nc.gpsimd.tensor_mul(
                            sl, sl, mask_sb[:, s * 128:(s + 1) * 128]
                        )

            def av(s, m, ks):
                for k in ks:
                    g, qo = k // 4, (k % 4) * 128
                    nc.tensor.matmul(
                        zslice(k),
                        at[(s, m, g)][:, qo:qo + 128],
                        v_quad[(s, m // 4)][:, m % 4, :],
                        start=False,
                        stop=(s == 1 and m == k),
                        skip_group_check=True,
                    )

            def finish(k):
                zp = zslice(k)
                rcp = work.tile([128, 1], F32, tag="rcp")
                nc.vector.reciprocal(rcp[:], zp[:, DV:DV + 1])
                nc.vector.tensor_scalar_mul(
                    z_out[k // 4][:, k % 4, :], zp[:, 0:DV], rcp[:]
                )

            def drain(g):
                # z_out[g] rows p, cols (s, v) -> out rows (4g+s)*128 + p
                dst = out_ext[g * 512:(g + 1) * 512, :].rearrange(
                    "(s p) v -> p s v", p=128
                )
                nc.sync.dma_start(out=dst, in_=z_out[g][:])

            # ---- emission: software-pipelined for PE continuity ----
            # The PE de-ramps (2.4 -> 1.2/0.65 GHz) whenever it idles, so
            # every consumer is emitted >= one PE phase after its producer:
            # DVE casts, gpsimd masks and ACT exps run in the shadow of
            # unrelated matmuls, and the head is filled with xq0-dependent
            # work so the PE never waits on the input DMA stream.
            proj("wq", xq_sb[0], qt[0], 0, "q")
            proj("wk", xq_sb[0], kt[(0, 0)], 0, "k")
            proj("wv", xq_sb[0], vt[(0, 0)], 0, "v")
            scores(0, [0, 1], 0)                      # kt00 cast done in V00
            scores(0, [2, 3], 0)
            vtrans(0, 0)                              # vt00 cast done above
            proj("wq", xq_sb[1], qt[1], 1, "q")       # xq1 lands ~6us
            av(0, 0, range(0, 4))
            av(0, 1, range(1, 4))
            proj("wk", xq_sb[1], kt[(0, 1)], 1, "k")
            av(0, 2, range(2, 4))
            av(0, 3, range(3, 4))
            proj("wv", xq_sb[1], vt[(0, 1)], 1, "v")
            scores(0, [0, 1], 1)                      # qt1 copied during K01
            vtrans(0, 1)                              # vt01 cast done above
            scores(0, [2, 3], 1)
            av(0, 0, range(4, 8))
            av(0, 1, range(4, 8))
            scores(0, [4, 5], 1)                      # kt01 cast done above
            av(0, 2, range(4, 8))
            av(0, 3, range(4, 8))
            scores(0, [6, 7], 1)
            av(0, 4, range(4, 8))
            av(0, 5, range(5, 8))
            proj("wk", xo_sb[0], kt[(1, 0)], 0, "k")  # xo0 lands ~9us
            av(0, 6, range(6, 8))
            av(0, 7, range(7, 8))
            scores(1, [0, 1], 0)                      # kt10 cast done above
            proj("wv", xo_sb[0], vt[(1, 0)], 0, "v")
            scores(1, [2, 3], 0)
            vtrans(1, 0)                              # vt10 cast done above
            proj("wk", xo_sb[1], kt[(1, 1)], 1, "k")  # spacer for DVE/ACT
            av(1, 0, range(0, 4))
            finish(0)
            av(1, 1, range(1, 4))
            finish(1)
            scores(1, [0, 1], 1)
            av(1, 2, range(2, 4))
            finish(2)
            av(1, 3, range(3, 4))
            finish(3)
            drain(0)
            proj("wv", xo_sb[1], vt[(1, 1)], 1, "v")
            scores(1, [2, 3], 1)
            av(1, 0, range(4, 8))
            av(1, 1, range(4, 8))
            vtrans(1, 1)                              # vt11 cast done above
            scores(1, [4, 5], 1)                      # kt11 cast done earlier
            av(1, 2, range(4, 8))
            av(1, 3, range(4, 8))
            scores(1, [6, 7], 1)
            av(1, 4, range(4, 8))
            av(1, 5, range(5, 8))
            av(1, 6, range(6, 8))
            av(1, 7, range(7, 8))
            for k in range(4, 8):
                finish(k)
            drain(1)

    nc.finalize()
    return nc


_NC = None


def _get_nc():
    global _NC
    if _NC is None:
        _NC = build_nc()
    return _NC


def _make_masks():
    npdt = _np_cdt()
    p = np.arange(128)[:, None]   # key (partition)
    q = np.arange(128)[None, :]   # query (free)
    triT = (p <= q).astype(np.float32)
    ones = np.ones((128, 128), np.float32)
    zero = np.zeros((128, 128), np.float32)
    # col block 0: own-parity key-slot m == k (diagonal, both parities);
    # col block 1: other-parity key-slot m == k (all-masked on even cores,
    # all-valid on odd cores)
    mask_even = np.concatenate([triT, zero], axis=1).astype(npdt)
    mask_odd = np.concatenate([triT, ones], axis=1).astype(npdt)
    return mask_even, mask_odd


def kernel(X, W_Q, W_K, W_V):
    X = np.asarray(X, np.float32)
    W_Q = np.asarray(W_Q, np.float32)
    W_K = np.asarray(W_K, np.float32)
    W_V = np.asarray(W_V, np.float32)

    nc = _get_nc()
    npdt = _np_cdt()
    mask_even, mask_odd = _make_masks()

    def warr(W):
        return np.ascontiguousarray(
            W.astype(npdt).reshape(NCH, 128, DK).transpose(1, 0, 2)
            .reshape(128, NCH * DK)
        )

    wq = warr(W_Q)
    wk = warr(W_K)
    wv = warr(W_V)

    def xarr(xt, cols):
        # xt [DM, L]; cols: slot-ordered column index (1024 entries)
        # -> [G, 128, NCH*512] with [g, p, c*512 + j] = xt[c*128+p, cols[g*512+j]]
        xs = xt[:, cols]                                   # [1024, 1024]
        return np.ascontiguousarray(
            xs.reshape(NCH, 128, G, 512).transpose(2, 1, 0, 3)
            .reshape(G, 128, NCH * 512)
        )

    in_maps = []
    for c in range(8):
        b, par = c // 2, c % 2
        xt_np = np.ascontiguousarray(X[b].T).astype(npdt)
        qcols = np.concatenate(
            [np.arange((2 * k + par) * 128, (2 * k + par + 1) * 128)
             for k in range(SLOTS)]
        )
        ocols = np.concatenate(
            [np.arange((2 * k + 1 - par) * 128, (2 * k + 2 - par) * 128)
             for k in range(SLOTS)]
        )
        in_maps.append({
            "xq": xarr(xt_np, qcols),
            "xo": xarr(xt_np, ocols),
            "wq": wq, "wk": wk, "wv": wv,
            "maskT": mask_odd if par else mask_even,
        })

    res = run_bass_kernel_spmd(nc, in_maps, list(range(8)))

    Z = np.zeros((B, L, DV), np.float32)
    for c in range(8):
        b, par = c // 2, c % 2
        o = res.results[c]["out"]
        for k in range(SLOTS):
            j = 2 * k + par
            Z[b, j * 128:(j + 1) * 128, :] = o[k * 128:(k + 1) * 128, :]
    return Z


# revision 14
# speedup vs baseline: 1.0064x; 1.0064x over previous
"""Causal attention (B=4, L=2048, d_model=1024, d_k=d_v=128) on 8 TRN2 NeuronCores.

Sharding (SPMD — one program, per-core data):
  core c -> batch b = c//2, parity par = c%2.
  Core handles q-blocks j = 2k+par for slot k in 0..7 (128 rows each).
  X^T's column blocks are split by parity into two slot-ordered inputs
  (xq = own query-parity blocks, xo = other parity), stored HOST-SIDE in
  piece-major [group, partition, chunk, col] layout so every input DMA is
  fully contiguous per partition (8KB runs).  Slot k attends key-slots
  0..k of EACH parity; the causal boundary is uniform (diagonal mask on
  own-parity key-slot m == k; other-parity m == k all-or-nothing by core
  parity) and is applied as a multiplicative 0/1 bf16 mask AFTER exp.

Within a core (all matmuls contract on the partition dim):
  - Projections: weight-stationary, one N=512 matmul per d_model chunk
    per 512-column group (no narrow pieces -> fewer LDWEIGHTS).
  - Scores TRANSPOSED: S^T[key, q] = K^T_blk.T @ Q^T; exp() writes A^T
    straight to SBUF (bf16).  Masking multiplies the diagonal q-slot
    after exp (scores bounded ~|12|, exp finite, so 0-mult is exact).
  - V transposes are quad-batched: 4 PE transposes into one PSUM bank,
    one strided copy out to a [128, 4, 129] tile whose col 128 is ones.
  - AV streams: as each A^T block lands, it is accumulated into the
    per-slot Z PSUM accumulators (8 slots packed 3-per-bank, 129 cols
    each: col 128 is the softmax denominator via the ones column).
    Slot k closes on (parity 1, m == k); outputs drain in two batched
    DMAs of 4 slots each.
"""

import os
import sys

sys.path.insert(0, "/opt/trn_rl_repo")
sys.path.insert(0, "/opt/trn_rl_repo/concourse")

import ml_dtypes
import numpy as np

import concourse.bass as bass  # noqa: F401
import concourse.mybir as mybir
import concourse.tile as tile
from concourse import bacc
from concourse.bass_utils import run_bass_kernel_spmd
from concourse.masks import make_identity

B, L, DM, DK, DV = 4, 2048, 1024, 128, 128
NB = L // 128    # 16 key blocks per batch
SLOTS = 8        # q-blocks per core
NCH = DM // 128  # 8 d_model chunks
G = 2            # q-slot groups of 4 (512 cols)
SCALE = float(DK) ** -0.5

COMPUTE = os.environ.get("ATTN_COMPUTE", "bf16")  # "bf16" | "f32"

F32 = mybir.dt.float32


def _cdt():
    return mybir.dt.bfloat16 if COMPUTE == "bf16" else mybir.dt.float32


def _np_cdt():
    return ml_dtypes.bfloat16 if COMPUTE == "bf16" else np.float32


def build_nc():
    cdt = _cdt()
    nc = bacc.Bacc()

    xq_ext = nc.declare_dram_parameter("xq", [G, 128, NCH * 512], cdt, isOutput=False)
    xo_ext = nc.declare_dram_parameter("xo", [G, 128, NCH * 512], cdt, isOutput=False)
    # weights pre-arranged on host to the SBUF chunk layout
    # [p, c*128+d] = W[c*128+p, d] so the DMA is fully contiguous
    wq_ext = nc.declare_dram_parameter("wq", [128, DM], cdt, isOutput=False)
    wk_ext = nc.declare_dram_parameter("wk", [128, DM], cdt, isOutput=False)
    wv_ext = nc.declare_dram_parameter("wv", [128, DM], cdt, isOutput=False)
    # multiplicative post-exp masks: [key 128, 2*128 q] — col block 0 is the
    # diagonal triangle (own parity), col block 1 all-0/all-1 (other parity)
    mask_ext = nc.declare_dram_parameter("maskT", [128, 256], cdt, isOutput=False)
    out_ext = nc.declare_dram_parameter("out", [SLOTS * 128, DV], F32, isOutput=True)

    xq_r = xq_ext.rearrange("g p (c j) -> g p c j", j=512)
    xo_r = xo_ext.rearrange("g p (c j) -> g p c j", j=512)

    with tile.TileContext(nc) as tc:
        with (
            tc.tile_pool(name="persist", bufs=1) as persist,
            tc.tile_pool(name="mm_ps", bufs=2, space="PSUM") as mm_ps,
            tc.tile_pool(name="st_ps", bufs=3, space="PSUM") as st_ps,
            tc.tile_pool(name="z_ps", bufs=1, space="PSUM") as z_ps,
            tc.tile_pool(name="work", bufs=4) as work,
        ):
            ident = persist.tile([128, 128], cdt, tag="ident")
            make_identity(nc, ident)
            # warm the ACT exp table during the initial DMA wait so the
            # first real exp doesn't pay the ~1.4us table load
            warm = work.tile([128, 1], F32, tag="warm")
            nc.scalar.activation(
                warm[:], ident[:, 0:1], mybir.ActivationFunctionType.Exp,
                bias=0.0, scale=1.0,
            )

            # ---- DMA issue (sync queue), in consumption order ----
            w_sb = {}

            def load_w(name, ext):
                t = persist.tile([128, NCH, 128], cdt, tag=name, name=name)
                nc.sync.dma_start(out=t[:], in_=ext.rearrange("p (c d) -> p c d", d=128))
                w_sb[name] = t

            # inputs split across both hwdge rings (sync + scalar) so the
            # two DMA queues stream in parallel; each ring's order matches
            # first-use time.
            def load_x(r, g, nm, eng, split=False):
                # returns list of (tile, base_chunk)
                if not split:
                    t = persist.tile([128, NCH, 512], cdt, tag=nm, name=nm)
                    eng.dma_start(out=t[:], in_=r[g])
                    return [(t, 0)]
                parts = []
                for h in range(2):
                    e = eng if h == 0 else nc.scalar
                    t = persist.tile([128, NCH // 2, 512], cdt,
                                     tag=f"{nm}_{h}", name=f"{nm}_{h}")
                    e.dma_start(
                        out=t[:], in_=r[g][:, h * (NCH // 2):(h + 1) * (NCH // 2), :]
                    )
                    parts.append((t, h * (NCH // 2)))
                return parts

            load_w("wq", wq_ext)
            xq_sb = [load_x(xq_r, 0, "xq0", nc.sync, split=True), None]
            load_w("wk", wk_ext)
            wv_sb = persist.tile([128, NCH, 128], cdt, tag="wv", name="wv")
            nc.scalar.dma_start(out=wv_sb[:], in_=wv_ext.rearrange("p (c d) -> p c d", d=128))
            w_sb["wv"] = wv_sb
            mask_sb = persist.tile([128, 256], cdt, tag="mask")
            nc.scalar.dma_start(out=mask_sb[:], in_=mask_ext[:])
            xq_sb[1] = load_x(xq_r, 1, "xq1", nc.sync)
            xo_sb = [load_x(xo_r, 0, "xo0", nc.sync),
                     load_x(xo_r, 1, "xo1", nc.scalar)]

            # ---- persistent SBUF tiles ----
            qt = [persist.tile([128, 512], cdt, tag=f"qt{g}", name=f"qt{g}")
                  for g in range(G)]
            kt = {(s, g): persist.tile([128, 512], cdt, tag=f"kt{s}{g}",
                                       name=f"kt{s}{g}")
                  for s in range(2) for g in range(G)}
            vt = {(s, g): persist.tile([128, 512], cdt, tag=f"vt{s}{g}",
                                       name=f"vt{s}{g}")
                  for s in range(2) for g in range(G)}
            # V in [key, v] layout, 4 blocks per tile, col 128 = ones
            v_quad = {}
            for s in range(2):
                for g in range(G):
                    t = persist.tile([128, 4, DV + 1], cdt, tag=f"vq{s}{g}",
                                     name=f"vq{s}{g}")
                    nc.vector.memset(t[:, :, DV:DV + 1], 1.0)
                    v_quad[(s, g)] = t
            at = {}
            for s in range(2):
                for m in range(SLOTS):
                    for g in range(G):
                        if m <= 4 * g + 3:
                            at[(s, m, g)] = persist.tile(
                                [128, 512], cdt, tag=f"at{s}_{m}_{g}",
                                name=f"at{s}_{m}_{g}")

            # z accumulators: 8 slots x [128, 129] f32 packed 3-per-bank.
            # A matmul with start=True aborts the bank's open accumulation
            # group, so slots sharing a bank must never use start: pre-zero
            # the banks and make every AV matmul a pure accumulate.
            zbank = []
            for i in range(3):
                t = z_ps.tile([128, 512], F32, tag=f"zb{i}", name=f"zb{i}")
                nc.vector.memset(t[:], 0.0)
                zbank.append(t)

            def zslice(k):
                b, o = k // 3, (k % 3) * (DV + 1)
                return zbank[b][:, o:o + DV + 1]

            # batched output staging: [128, 4 slots * 128]
            z_out = [persist.tile([128, 4, DV], F32, tag=f"zo{g}", name=f"zo{g}")
                     for g in range(G)]

            # ---- stages ----
            def proj(name, src, dst, g, kind):
                w = w_sb[name]
                ps = mm_ps.tile([128, 512], F32, tag="mm", name=f"p_{name}{g}")
                for t, base in src:
                    nch = t.shape[1]
                    for i in range(nch):
                        c = base + i
                        nc.tensor.matmul(
                            ps[:], w[:, c, :], t[:, i, :],
                            start=(c == 0), stop=(c == NCH - 1),
                        )
                if kind == "q":
                    # DVE, not ACT: keeps the scalar engine exp-only and
                    # recycles the proj psum buffer fast
                    nc.vector.tensor_scalar_mul(dst[:], ps[:], SCALE)
                else:
                    nc.vector.tensor_copy(dst[:], ps[:])

            def vtrans(s, g):
                tq = mm_ps.tile([128, 4, 128], cdt, tag="mm", name=f"tq{s}{g}")
                for i in range(4):
                    nc.tensor.transpose(
                        tq[:, i, :], vt[(s, g)][:, i * 128:(i + 1) * 128], ident[:]
                    )
                nc.vector.tensor_copy(v_quad[(s, g)][:, :, 0:DV], tq[:])

            def scores(s, ms, g):
                for m in ms:
                    a = max(m - 4 * g, 0)
                    st = st_ps.tile([128, 512], F32, tag="st", name=f"st{s}{m}{g}")
                    nc.tensor.matmul(
                        st[:, a * 128:512],
                        kt[(s, m // 4)][:, (m % 4) * 128:(m % 4 + 1) * 128],
                        qt[g][:, a * 128:512],
                        start=True, stop=True,
                        skip_group_check=True,
                    )
                    nc.scalar.activation(
                        at[(s, m, g)][:, a * 128:512],
                        st[:, a * 128:512],
                        mybir.ActivationFunctionType.Exp,
                        bias=0.0, scale=1.0,
                    )
                    if 4 * g <= m <= 4 * g + 3:
                        # multiplicative causal mask on the idle gpsimd
                        # engine, keeping DVE free for psum casts
                        qo = (m - 4 * g) * 128
                        sl = at[(s, m, g)][:, qo:qo + 128]
                        nc.gpsimd.tensor_mul(
                            sl, sl, mask_sb[:, s * 128:(s + 1) * 128]
                        )

            def av(s, m, ks):
                for k in ks:
                    g, qo = k // 4, (k % 4) * 128
                    nc.tensor.matmul(
                        zslice(k),
                        at[(s, m, g)][:, qo:qo + 128],
                        v_quad[(s, m // 4)][:, m % 4, :],
                        start=False,
                        stop=(s == 1 and m == k),
                        skip_group_check=True,
                    )

            def finish(k):
                zp = zslice(k)
                rcp = work.tile([128, 1], F32, tag="rcp")
                nc.vector.reciprocal(rcp[:], zp[:, DV:DV + 1])
                nc.vector.tensor_scalar_mul(
                    z_out[k // 4][:, k % 4, :], zp[:, 0:DV], rcp[:]
                )

            def drain(g):
                # z_out[g] rows p, cols (s, v) -> out rows (4g+s)*128 + p
                dst = out_ext[g * 512:(g + 1) * 512, :].rearrange(
                    "(s p) v -> p s v", p=128
                )
                nc.sync.dma_start(out=dst, in_=z_out[g][:])

            # ---- emission: software-pipelined for PE continuity ----
            # The PE de-ramps (2.4 -> 1.2/0.65 GHz) whenever it idles, so
            # every consumer is emitted >= one PE phase after its producer:
            # DVE casts, gpsimd masks and ACT exps run in the shadow of
            # unrelated matmuls, and the head is filled with xq0-dependent
            # work so the PE never waits on the input DMA stream.
            proj("wq", xq_sb[0], qt[0], 0, "q")
            proj("wk", xq_sb[0], kt[(0, 0)], 0, "k")
            proj("wv", xq_sb[0], vt[(0, 0)], 0, "v")
            scores(0, [0, 1], 0)                      # kt00 cast done in V00
            scores(0, [2, 3], 0)
            vtrans(0, 0)                              # vt00 cast done above
            proj("wq", xq_sb[1], qt[1], 1, "q")       # xq1 lands ~6us
            av(0, 0, range(0, 4))
            av(0, 1, range(1, 4))
            proj("wk", xq_sb[1], kt[(0, 1)], 1, "k")
            av(0, 2, range(2, 4))
            av(0, 3, range(3, 4))
            proj("wv", xq_sb[1], vt[(0, 1)], 1, "v")
            scores(0, [0, 1], 1)                      # qt1 copied during K01
            vtrans(0, 1)                              # vt01 cast done above
            scores(0, [2, 3], 1)
            av(0, 0, range(4, 8))
            av(0, 1, range(4, 8))
            scores(0, [4, 5], 1)                      # kt01 cast done above
            av(0, 2, range(4, 8))
            av(0, 3, range(4, 8))
            scores(0, [6, 7], 1)
            av(0, 4, range(4, 8))
            av(0, 5, range(5, 8))
            proj("wk", xo_sb[0], kt[(1, 0)], 0, "k")  # xo0 lands ~9us
            av(0, 6, range(6, 8))
            av(0, 7, range(7, 8))
            scores(1, [0, 1], 0)                      # kt10 cast done above
            proj("wv", xo_sb[0], vt[(1, 0)], 0, "v")
            scores(1, [2, 3], 0)
            vtrans(1, 0)                              # vt10 cast done above
            proj("wk", xo_sb[1], kt[(1, 1)], 1, "k")  # spacer for DVE/ACT
            av(1, 0, range(0, 4))
            finish(0)
            av(1, 1, range(1, 4))
            finish(1)
            scores(1, [0, 1], 1)
            av(1, 2, range(2, 4))
            finish(2)
            av(1, 3, range(3, 4))
            finish(3)
            drain(0)
            proj("wv", xo_sb[1], vt[(1, 1)], 1, "v")
            scores(1, [2, 3], 1)
            av(1, 0, range(4, 8))
            av(1, 1, range(4, 8))
            vtrans(1, 1)                              # vt11 cast done above
            scores(1, [4, 5], 1)                      # kt11 cast done earlier
            av(1, 2, range(4, 8))
            av(1, 3, range(4, 8))
            scores(1, [6, 7], 1)
            av(1, 4, range(4, 8))
            av(1, 5, range(5, 8))
            av(1, 6, range(6, 8))
            av(1, 7, range(7, 8))
            for k in range(4, 8):
                finish(k)
            drain(1)

    nc.finalize()
    return nc


_NC = None


def _get_nc():
    global _NC
    if _NC is None:
        _NC = build_nc()
    return _NC


def _make_masks():
    npdt = _np_cdt()
    p = np.arange(128)[:, None]   # key (partition)
    q = np.arange(128)[None, :]   # query (free)
    triT = (p <= q).astype(np.float32)
    ones = np.ones((128, 128), np.float32)
    zero = np.zeros((128, 128), np.float32)
    # col block 0: own-parity key-slot m == k (diagonal, both parities);
    # col block 1: other-parity key-slot m == k (all-masked on even cores,
    # all-valid on odd cores)
    mask_even = np.concatenate([triT, zero], axis=1).astype(npdt)
    mask_odd = np.concatenate([triT, ones], axis=1).astype(npdt)
    return mask_even, mask_odd


def kernel(X, W_Q, W_K, W_V):
    X = np.asarray(X, np.float32)
    W_Q = np.asarray(W_Q, np.float32)
    W_K = np.asarray(W_K, np.float32)
    W_V = np.asarray(W_V, np.float32)

    nc = _get_nc()
    npdt = _np_cdt()
    mask_even, mask_odd = _make_masks()

    def warr(W):
        return np.ascontiguousarray(
            W.astype(npdt).reshape(NCH, 128, DK).transpose(1, 0, 2)
            .reshape(128, NCH * DK)
        )

    wq = warr(W_Q)
    wk = warr(W_K)
    wv = warr(W_V)

    def xarr(xt, cols):
        # xt [DM, L]; cols: slot-ordered column index (1024 entries)
        # -> [G, 128, NCH*512] with [g, p, c*512 + j] = xt[c*128+p, cols[g*512+j]]
        xs = xt[:, cols]                                   # [1024, 1024]
        return np.ascontiguousarray(
            xs.reshape(NCH, 128, G, 512).transpose(2, 1, 0, 3)
            .reshape(G, 128, NCH * 512)
        )

    in_maps = []
    for c in range(8):
        b, par = c // 2, c % 2
        xt_np = np.ascontiguousarray(X[b].T).astype(npdt)
        qcols = np.concatenate(
            [np.arange((2 * k + par) * 128, (2 * k + par + 1) * 128)
             for k in range(SLOTS)]
        )
        ocols = np.concatenate(
            [np.arange((2 * k + 1 - par) * 128, (2 * k + 2 - par) * 128)
             for k in range(SLOTS)]
        )
        in_maps.append({
            "xq": xarr(xt_np, qcols),
            "xo": xarr(xt_np, ocols),
            "wq": wq, "wk": wk, "wv": wv,
            "maskT": mask_odd if par else mask_even,
        })

    res = run_bass_kernel_spmd(nc, in_maps, list(range(8)))

    Z = np.zeros((B, L, DV), np.float32)
    for c in range(8):
        b, par = c // 2, c % 2
        o = res.results[c]["out"]
        for k in range(SLOTS):
            j = 2 * k + par
            Z[b, j * 128:(j + 1) * 128, :] = o[k * 128:(k + 1) * 128, :]
    return Z
